# revision 7
# baseline (speedup 1.0000x reference)
"""AttentionPool (segment softmax-pool) Trainium2 kernel.

Math (matches reference up to per-segment-constant invariance of softmax):
    h    = relu(x @ W1 + b1)                [N, 64]
    gate = h @ W2 (+ b2, dropped: constant) [N]
    alpha = segment_softmax(gate, batch)    [N]   (max-subtraction dropped:
                                                   gate is O(1), exp safe)
    out[g] = sum_{batch[i]==g} alpha[i] * x[i]    [G, 128]

Precision strategy: the weighted-sum copy of x (xn) is fp16 with fp32
PSUM accumulation; the gate-path copy (xt, transposed on host) is
fp8-e4m3 -- the gate only steers the softmax, and fp8 there measures
~6e-3 output rel-err, well inside the 2e-2 budget.

Perf structure (v2; the 421-507us v1 was limited by per-instruction
fixed costs on ACT/DVE (~220ns each on small ops) and a serialized
mm1->relu->mm2->exp->E->po chain, with DMA queues at ~50% duty):
  - DMA: xn groups split 3:1 between the sync and scalar HWDGE rings
    so both queues carry ~100KB/supertile; xt+mask ride scalar. Inputs
    prefetched 3 groups (48 supertiles) ahead, quad-buffered.
  - relu+bias batched x2 supertiles ([128,512], alternating ACT/DVE),
    exp batched x4 ([128,16]), mask*e batched x4 on GPSIMD -- amortizes
    the ~220ns fixed cost per activation-engine instruction.
  - po: the 4 per-chunk E.T@xn matmuls of a supertile go to 4 disjoint
    32-column groups of the PE array (tile_position (0,32k) via psum
    base partition), so they stream CONCURRENTLY: ~70ns vs 4x59ns.
    NG<=32 slots per supertile (actual data: 4).
  - po evacuation [128,2,129] per 2 supertiles alternating ACT/DVE;
    out DMA per 8 supertiles as 4 strip transfers on sync.
  - Emission is a 3-stage software pipeline over 4-supertile blocks:
    stage A = mm1+relu of block b, stage B = mm2+exp+E of block b-1,
    stage C = po+evac+out of block b-2 -- so the PE FIFO never waits
    on the scalar-engine chain of the same block.

Device pipeline per 512-node supertile (per core, nodes split across 8):
    mm1 x2 (col-concurrent): ph [128, 2, 256] psum <- w1p-half.T @ xt-half
    relu+bias x2-batch -> h fp16 [128, 2, 256]  (ACT / DVE alternating)
    mm2 x4: lhsT=h-quadrant [64,128], rhs=w2dup-half -> pg4 [128, 4, 4]
    (x4-batch) ACT exp -> e4 [128, 4, 4] f32
    (x4-batch) GPSIMD: E4 [128, 4, 4, NG] f16 = host_mask(u8) * e4-bcast
    po x4 col-strips: psum po[32k:32k+NG, t%2, :] = E4_k.T @ xn_k
    (every 2 st) ACT/DVE copy po pair -> SBUF [128, 8, 129] f16 slot
    (every 8 st) out-DMA x4 strips [NG, 8, 129] f16 (sync q)
Host scatter-adds the 4xNG strip partials into [G,129] and divides.
"""

import time

import numpy as np
import ml_dtypes
from contextlib import ExitStack

import concourse.bass as bass
import concourse.tile as tile
from concourse import bacc, mybir
from concourse.bass_utils import run_bass_kernel_spmd

F32 = mybir.dt.float32
F16 = mybir.dt.float16
F8 = mybir.dt.float8e4
U8 = mybir.dt.uint8
NP_F8 = ml_dtypes.float8_e4m3

CORES = 8
D = 128
HID = 64
G_SEGMENTS = 8192
SUB = 128
KSUB = 4
SUPER = SUB * KSUB  # 512
DW = D + 1  # x row + ones column
GROUP = 16  # supertiles per input DMA
MGROUP = 64  # supertiles per mask DMA
OB = 8  # supertiles per output DMA
PB = 2  # supertiles per po PSUM bank / evacuation copy
EB = 4  # supertiles per exp/E batch (one pipeline block)
RB = 2  # supertiles per relu batch


def build_program(T: int, NG: int):
    """Build the per-core Bass program (same program for all 8 cores)."""
    assert T % GROUP == 0 and GROUP % OB == 0 and OB % EB == 0 and EB % PB == 0
    assert MGROUP % EB == 0 and NG <= 32 and T % EB == 0
    nc = bacc.Bacc(None, target_bir_lowering=False)

    TG = T // GROUP
    xn_d = nc.dram_tensor("xn", [TG, SUB, GROUP, KSUB, DW], F16, kind="ExternalInput")
    xt_d = nc.dram_tensor("xt", [TG, D, GROUP, SUPER], F8, kind="ExternalInput")
    mask_d = nc.dram_tensor("mask", [-(-T // MGROUP), SUB, MGROUP, KSUB, NG], U8, kind="ExternalInput")
    w1_d = nc.dram_tensor("w1p", [D, D], F16, kind="ExternalInput")
    b1_d = nc.dram_tensor("b1d", [D, 1], F32, kind="ExternalInput")
    w2_d = nc.dram_tensor("w2d", [D, 1], F16, kind="ExternalInput")
    out_d = nc.dram_tensor(
        "out_part", [T // OB, KSUB, NG, OB, DW], F16, kind="ExternalOutput"
    )

    with ExitStack() as ctx:
        tc = ctx.enter_context(tile.TileContext(nc))
        consts = ctx.enter_context(tc.tile_pool(name="consts", bufs=1))
        # xn is consumed by stage C two blocks late: bufs must cover
        # groups g-1..g+4 (6) so the prefetch DMA's buffer-reuse target
        # has all its consumers already emitted (else: WAR race).
        xnpool = ctx.enter_context(tc.tile_pool(name="xnpool", bufs=6))
        xtpool = ctx.enter_context(tc.tile_pool(name="xtpool", bufs=5))
        mpool = ctx.enter_context(tc.tile_pool(name="mpool", bufs=2))
        hpool = ctx.enter_context(tc.tile_pool(name="hpool", bufs=4))
        epool = ctx.enter_context(tc.tile_pool(name="epool", bufs=3))
        opool = ctx.enter_context(tc.tile_pool(name="opool", bufs=2))
        ps_h = ctx.enter_context(
            tc.tile_pool(name="ps_h", bufs=3, space=bass.MemorySpace.PSUM)
        )
        ps_g = ctx.enter_context(
            tc.tile_pool(name="ps_g", bufs=2, space=bass.MemorySpace.PSUM)
        )
        ps_o = ctx.enter_context(
            tc.tile_pool(name="ps_o", bufs=2, space=bass.MemorySpace.PSUM)
        )

        w1p = consts.tile([D, D], F16)
        nc.sync.dma_start(w1p, w1_d[:, :])
        b1d = consts.tile([D, 1], F32)
        nc.sync.dma_start(b1d, b1_d[:, :])
        w2d = consts.tile([D, 1], F16)
        nc.sync.dma_start(w2d, w2_d[:, :])

        tiles = {}
        mtiles = {}
        hs = {}  # pair index -> h SBUF tile
        E4s = {}  # block index -> E4 SBUF tile
        st = {"ph": None, "pg4": None, "po": None, "po_sb": None}

        def issue_group(g):
            xng = xnpool.tile([SUB, GROUP, KSUB, DW], F16, tag="xn")
            # 3:1 sync/scalar split keeps both HWDGE rings near-equal bytes
            eng = nc.scalar if g % 4 == 3 else nc.sync
            eng.dma_start(xng, xn_d[g])
            xtg = xtpool.tile([D, GROUP, SUPER], F8, tag="xt")
            nc.scalar.dma_start(xtg, xt_d[g])
            tiles[g] = (xng, xtg)
            if g % (MGROUP // GROUP) == 0:
                m_sb = mpool.tile([SUB, MGROUP, KSUB, NG], U8, tag="mask")
                nc.scalar.dma_start(m_sb, mask_d[g * GROUP // MGROUP])
                mtiles[g * GROUP // MGROUP] = m_sb

        def stage_a(b):
            """mm1 pairs + relu for the 4 supertiles of block b."""
            for j in range(EB):
                t = b * EB + j
                g, gi = divmod(t, GROUP)
                if gi == 1 and g + 4 < TG:
                    issue_group(g + 4)
                xt = tiles[g][1][:, gi]  # [D, SUPER]
                # packed mm1 (col-concurrent halves):
                # ph[0:64,p,c] = h(node c), ph[64:128,p,c] = h(node 256+c)
                if t % RB == 0:
                    st["ph"] = ps_h.tile([D, RB, 256], F32, name="ph")
                ph = st["ph"]
                nc.tensor.matmul(
                    ph[0:HID, t % RB, :], w1p[:, 0:HID], xt[:, 0:256],
                    start=True, stop=True,
                )
                nc.tensor.matmul(
                    ph[HID:D, t % RB, :], w1p[:, HID:D], xt[:, 256:512],
                    start=True, stop=True,
                )
                if t % RB == RB - 1:
                    p = t // RB
                    h = hpool.tile([D, RB, 256], F16, name="h")
                    if p % 2 == 0:
                        nc.scalar.activation(
                            h, ph, mybir.ActivationFunctionType.Relu,
                            bias=b1d, scale=1.0,
                        )
                    else:
                        nc.vector.tensor_scalar(
                            h, ph, b1d, 0.0, mybir.AluOpType.add, mybir.AluOpType.max
                        )
                    hs[p] = h

        def stage_b(b):
            """mm2 quads + exp + E multiply for block b."""
            pg4 = ps_g.tile([SUB, EB, KSUB], F32, name="pg4")
            for j in range(EB):
                t = b * EB + j
                h = hs[t // RB]
                hj = t % RB
                for k in range(KSUB):
                    r0 = HID * (k // 2)
                    c0 = SUB * (k % 2)
                    nc.tensor.matmul(
                        pg4[:, j, k : k + 1],
                        h[r0 : r0 + HID, hj, c0 : c0 + SUB],
                        w2d[r0 : r0 + HID, :],
                        start=True,
                        stop=True,
                    )
                if hj == RB - 1:
                    del hs[t // RB]
            e4 = epool.tile([SUB, EB, KSUB], F32, tag="e")
            nc.scalar.activation(e4, pg4, mybir.ActivationFunctionType.Exp)
            t0 = b * EB
            m_sb = mtiles[t0 // MGROUP]
            E4 = epool.tile([SUB, EB, KSUB, NG], F16, tag="E")
            nc.gpsimd.tensor_mul(
                E4,
                m_sb[:, t0 % MGROUP : t0 % MGROUP + EB],
                e4.to_broadcast([SUB, EB, KSUB, NG]),
            )
            E4s[b] = E4

        pending_out = []

        def flush_out():
            for ob_idx, sb in pending_out:
                for s4 in range(KSUB):
                    nc.sync.dma_start(out_d[ob_idx, s4], sb[32 * s4 : 32 * s4 + NG])
            pending_out.clear()

        def stage_c(b):
            """po strip-matmuls + evacuation + (delayed) out DMA for block b."""
            flush_out()
            E4 = E4s.pop(b)
            for j in range(EB):
                t = b * EB + j
                g, gi = divmod(t, GROUP)
                xng = tiles[g][0]
                if t % PB == 0:
                    st["po"] = ps_o.tile([SUB, PB, DW], F32, name="po")
                po = st["po"]
                for k in range(KSUB):
                    nc.tensor.matmul(
                        po[32 * k : 32 * k + NG, t % PB, :],
                        E4[:, j, k, :],
                        xng[:, gi, k, :],
                        start=True,
                        stop=True,
                        tile_position=(0, 32 * k),
                    )
                if t % OB == 0:
                    st["po_sb"] = opool.tile([SUB, OB, DW], F16, tag="po", name="po_sb")
                po_sb = st["po_sb"]
                if t % PB == PB - 1:
                    s = (t % OB) - PB + 1
                    if (t // PB) % 2 == 0:
                        nc.vector.tensor_copy(po_sb[:, s : s + PB, :], po)
                    else:
                        nc.scalar.activation(
                            po_sb[:, s : s + PB, :],
                            po,
                            mybir.ActivationFunctionType.Copy,
                        )
                if t % OB == OB - 1:
                    pending_out.append((t // OB, po_sb))
                if gi == GROUP - 1:
                    del tiles[g]

        for g0 in range(min(4, TG)):
            issue_group(g0)

        blocks = T // EB
        for b in range(blocks + 2):
            if b < blocks:
                stage_a(b)
            if 1 <= b <= blocks:
                stage_b(b - 1)
            if b >= 2:
                stage_c(b - 2)
        flush_out()

    nc.compile()
    return nc


def preprocess(x: np.ndarray, batch: np.ndarray):
    """Shard + pad inputs, cast x to fp16 (natural) + fp8 (transposed)
    device layouts (grouped for batched DMA), build per-supertile masks
    and graph-id tables."""
    N = x.shape[0]
    n_core = -(-N // CORES)
    npc = -(-n_core // (SUPER * GROUP)) * (SUPER * GROUP)
    T = npc // SUPER
    TG = T // GROUP

    xs = np.zeros((CORES, npc, D), np.float32)
    b_pad = np.empty((CORES, npc), np.int64)
    valid = np.zeros((CORES, npc), bool)
    for c in range(CORES):
        s, e = c * n_core, min((c + 1) * n_core, N)
        n = e - s
        xs[c, :n] = x[s:e]
        b_pad[c, :n] = batch[s:e] if n > 0 else 0
        b_pad[c, n:] = batch[e - 1] if n > 0 else 0
        valid[c, :n] = True

    f16 = np.float16
    x16 = xs.astype(f16)  # [C, npc, D]
    # natural layout, grouped: [C, TG, SUB, GROUP, KSUB, DW]
    xn = np.zeros((CORES, TG, SUB, GROUP, KSUB, DW), f16)
    x6 = x16.reshape(CORES, TG, GROUP, KSUB, SUB, D).transpose(0, 1, 4, 2, 3, 5)
    xn[..., :D] = x6
    xn[..., D] = f16(1.0)
    # transposed gate layout in fp8, grouped: [C, TG, D, GROUP, SUPER]
    xt = np.ascontiguousarray(
        xs.astype(NP_F8).reshape(CORES, TG, GROUP, SUPER, D).transpose(0, 1, 4, 2, 3)
    )

    v = b_pad.reshape(CORES, T, SUPER)
    chg = np.zeros(v.shape, bool)
    chg[..., 1:] = v[..., 1:] != v[..., :-1]
    loc = np.cumsum(chg, axis=-1)  # [C,T,SUPER] local distinct index
    NG = int(loc.max()) + 1
    NG = max(4, -(-NG // 4) * 4)

    vmask = valid.reshape(CORES, T, SUPER)
    onehot = (loc[..., None] == np.arange(NG)) & vmask[..., None]
    # [C,T,SUPER,NG] -> [C, ceil(T/MGROUP), SUB, MGROUP, KSUB, NG]
    TM2 = -(-T // MGROUP)
    mask = np.zeros((CORES, TM2 * MGROUP, KSUB, SUB, NG), np.uint8)
    mask[:, :T] = onehot.reshape(CORES, T, KSUB, SUB, NG)
    mask = np.ascontiguousarray(
        mask.reshape(CORES, TM2, MGROUP, KSUB, SUB, NG).transpose(
            0, 1, 4, 2, 3, 5
        )
    )

    # pad nodes have all-zero mask rows (zero partials), so they may share
    # the last real graph's id slot without corrupting it
    gids = np.zeros((CORES, T, NG), np.int64)
    cc, tt = np.meshgrid(np.arange(CORES), np.arange(T), indexing="ij")
    cc = cc[..., None] * np.ones((1, 1, SUPER), int)
    tt = tt[..., None] * np.ones((1, 1, SUPER), int)
    gids[cc.ravel(), tt.ravel(), loc.ravel()] = v.ravel()

    return xn, xt, mask, gids, T, NG


def _kernel_impl(x, batch, W1, b1, W2, b2=None, **run_kwargs):
    f16 = np.float16
    x = np.ascontiguousarray(np.asarray(x, dtype=np.float32))
    batch = np.asarray(batch).astype(np.int64)
    W1 = np.asarray(W1, dtype=np.float32).astype(f16)  # [D, HID]
    b1 = np.asarray(b1, dtype=np.float32).reshape(HID, 1)
    W2 = np.asarray(W2, dtype=np.float32).astype(f16).reshape(HID, 1)
    w1p = np.concatenate([W1, W1], axis=1)  # [D, D]
    b1d = np.concatenate([b1, b1], axis=0)  # [D, 1]
    w2d = np.concatenate([W2, W2], axis=0)  # [D, 1]

    xn, xt, mask, gids, T, NG = preprocess(x, batch)

    nc = build_program(T, NG)
    in_maps = [
        {
            "xn": xn[c],
            "xt": xt[c],
            "mask": mask[c],
            "w1p": w1p,
            "b1d": b1d,
            "w2d": w2d,
        }
        for c in range(CORES)
    ]
    # The axon/TRN device occasionally comes up wedged from a prior run
    # (NRT exec errors that clear after a few attempts) -- retry rather
    # than fail the whole call.
    last_err = None
    for attempt in range(4):
        try:
            res = run_bass_kernel_spmd(
                nc, in_maps, core_ids=list(range(CORES)), **run_kwargs
            )
            break
        except Exception as err:  # noqa: BLE001 - device-side transients
            last_err = err
            if attempt == 3:
                raise
            if "nrt_profile" in str(err):
                run_kwargs = {**run_kwargs, "trace": False}
            else:
                time.sleep(20.0)
    # [C, T//OB, KSUB, NG, OB, DW] -> [C, T, KSUB, NG, DW]
    parts = np.stack([r["out_part"] for r in res.results]).astype(np.float32)
    C = parts.shape[0]
    parts = parts.transpose(0, 1, 4, 2, 3, 5).reshape(C, T, KSUB, NG, DW)

    G = G_SEGMENTS
    acc = np.zeros((G + 1, DW), np.float32)
    idx = np.where(gids >= 0, gids, G)  # [C, T, NG]
    idx = np.broadcast_to(idx[:, :, None, :], (C, T, KSUB, NG)).ravel()
    np.add.at(acc, idx, parts.reshape(-1, DW))
    den = acc[:G, D]
    S = acc[:G, :D]
    out = np.where(den[:, None] > 0, S / np.maximum(den, 1e-30)[:, None], 0.0)
    return out.astype(np.float32), res


def kernel(x, batch, W1, b1, W2, b2):
    out, _ = _kernel_impl(x, batch, W1, b1, W2, b2)
    return out


# revision 8
# speedup vs baseline: 1.0001x; 1.0001x over previous
"""AttentionPool (segment softmax-pool) Trainium2 kernel.

Math (matches reference up to per-segment-constant invariance of softmax):
    h    = relu(x @ W1 + b1)                [N, 64]
    gate = h @ W2 (+ b2, dropped: constant) [N]
    alpha = segment_softmax(gate, batch)    [N]   (max-subtraction dropped:
                                                   gate is O(1), exp safe)
    out[g] = sum_{batch[i]==g} alpha[i] * x[i]    [G, 128]

Precision strategy: the weighted-sum copy of x (xn) is fp16 with fp32
PSUM accumulation; the gate-path copy (xt, transposed on host) is
fp8-e4m3 -- the gate only steers the softmax, and fp8 there measures
~6e-3 output rel-err, well inside the 2e-2 budget.

Perf structure (v2; the 421-507us v1 was limited by per-instruction
fixed costs on ACT/DVE (~220ns each on small ops) and a serialized
mm1->relu->mm2->exp->E->po chain, with DMA queues at ~50% duty):
  - DMA: xn groups split 3:1 between the sync and scalar HWDGE rings
    so both queues carry ~100KB/supertile; xt+mask ride scalar. Inputs
    prefetched 3 groups (48 supertiles) ahead, quad-buffered.
  - relu+bias batched x2 supertiles ([128,512], alternating ACT/DVE),
    exp batched x4 ([128,16]), mask*e batched x4 on GPSIMD -- amortizes
    the ~220ns fixed cost per activation-engine instruction.
  - po: the 4 per-chunk E.T@xn matmuls of a supertile go to 4 disjoint
    32-column groups of the PE array (tile_position (0,32k) via psum
    base partition), so they stream CONCURRENTLY: ~70ns vs 4x59ns.
    NG<=32 slots per supertile (actual data: 4).
  - po evacuation [128,2,129] per 2 supertiles alternating ACT/DVE;
    out DMA per 8 supertiles as 4 strip transfers on sync.
  - Emission is a 3-stage software pipeline over 4-supertile blocks:
    stage A = mm1+relu of block b, stage B = mm2+exp+E of block b-1,
    stage C = po+evac+out of block b-2 -- so the PE FIFO never waits
    on the scalar-engine chain of the same block.

Device pipeline per 512-node supertile (per core, nodes split across 8):
    mm1 x2 (col-concurrent): ph [128, 2, 256] psum <- w1p-half.T @ xt-half
    relu+bias x2-batch -> h fp16 [128, 2, 256]  (ACT / DVE alternating)
    mm2 x4: lhsT=h-quadrant [64,128], rhs=w2dup-half -> pg4 [128, 4, 4]
    (x4-batch) ACT exp -> e4 [128, 4, 4] f32
    (x4-batch) GPSIMD: E4 [128, 4, 4, NG] f16 = host_mask(u8) * e4-bcast
    po x4 col-strips: psum po[32k:32k+NG, t%2, :] = E4_k.T @ xn_k
    (every 2 st) ACT/DVE copy po pair -> SBUF [128, 8, 129] f16 slot
    (every 8 st) out-DMA x4 strips [NG, 8, 129] f16 (sync q)
Host scatter-adds the 4xNG strip partials into [G,129] and divides.
"""

import time

import numpy as np
import ml_dtypes
from contextlib import ExitStack

import concourse.bass as bass
import concourse.tile as tile
from concourse import bacc, mybir
from concourse.bass_utils import run_bass_kernel_spmd

F32 = mybir.dt.float32
F16 = mybir.dt.float16
F8 = mybir.dt.float8e4
U8 = mybir.dt.uint8
NP_F8 = ml_dtypes.float8_e4m3

CORES = 8
D = 128
HID = 64
G_SEGMENTS = 8192
SUB = 128
KSUB = 4
SUPER = SUB * KSUB  # 512
DW = D + 1  # x row + ones column
GROUP = 16  # supertiles per input DMA
MGROUP = 64  # supertiles per mask DMA
OB = 8  # supertiles per output DMA
PB = 2  # supertiles per po PSUM bank / evacuation copy
EB = 4  # supertiles per exp/E batch (one pipeline block)
RB = 2  # supertiles per relu batch


def build_program(T: int, NG: int):
    """Build the per-core Bass program (same program for all 8 cores)."""
    assert T % GROUP == 0 and GROUP % OB == 0 and OB % EB == 0 and EB % PB == 0
    assert MGROUP % EB == 0 and NG <= 32 and T % EB == 0
    nc = bacc.Bacc(None, target_bir_lowering=False)

    TG = T // GROUP
    xn_d = nc.dram_tensor("xn", [TG, SUB, GROUP, KSUB, DW], F16, kind="ExternalInput")
    xt_d = nc.dram_tensor("xt", [TG, D, GROUP, SUPER], F8, kind="ExternalInput")
    mask_d = nc.dram_tensor("mask", [-(-T // MGROUP), SUB, MGROUP, KSUB, NG], U8, kind="ExternalInput")
    w1_d = nc.dram_tensor("w1p", [D, D], F16, kind="ExternalInput")
    b1_d = nc.dram_tensor("b1d", [D, 1], F32, kind="ExternalInput")
    w2_d = nc.dram_tensor("w2d", [D, 1], F16, kind="ExternalInput")
    out_d = nc.dram_tensor(
        "out_part", [T // OB, KSUB, NG, OB, DW], F16, kind="ExternalOutput"
    )

    with ExitStack() as ctx:
        tc = ctx.enter_context(tile.TileContext(nc))
        consts = ctx.enter_context(tc.tile_pool(name="consts", bufs=1))
        # xn is consumed by stage C two blocks late: bufs must cover
        # groups g-1..g+4 (6) so the prefetch DMA's buffer-reuse target
        # has all its consumers already emitted (else: WAR race).
        xnpool = ctx.enter_context(tc.tile_pool(name="xnpool", bufs=6))
        xtpool = ctx.enter_context(tc.tile_pool(name="xtpool", bufs=5))
        mpool = ctx.enter_context(tc.tile_pool(name="mpool", bufs=2))
        hpool = ctx.enter_context(tc.tile_pool(name="hpool", bufs=4))
        epool = ctx.enter_context(tc.tile_pool(name="epool", bufs=3))
        opool = ctx.enter_context(tc.tile_pool(name="opool", bufs=2))
        ps_h = ctx.enter_context(
            tc.tile_pool(name="ps_h", bufs=3, space=bass.MemorySpace.PSUM)
        )
        ps_g = ctx.enter_context(
            tc.tile_pool(name="ps_g", bufs=2, space=bass.MemorySpace.PSUM)
        )
        ps_o = ctx.enter_context(
            tc.tile_pool(name="ps_o", bufs=2, space=bass.MemorySpace.PSUM)
        )
        ps_x = ctx.enter_context(
            tc.tile_pool(name="ps_x", bufs=1, space=bass.MemorySpace.PSUM)
        )

        w1p = consts.tile([D, D], F16)
        nc.sync.dma_start(w1p, w1_d[:, :])
        b1d = consts.tile([D, 1], F32)
        nc.sync.dma_start(b1d, b1_d[:, :])
        w2d = consts.tile([D, 1], F16)
        nc.sync.dma_start(w2d, w2_d[:, :])

        scratch = ps_x.tile([D, D], F32, name="scratch")

        def dummy(n):
            # Dependency-free filler matmuls: keep the PE HAM activity
            # monitor in the warm (K=8/8, 2.4 GHz) state across the
            # ~0.5-1us idle gaps of the DMA-bound steady state. Without
            # them the PE runs at 1.2 GHz nearly all kernel (v3 trace:
            # first warm transition at t=322us of a 462us span).
            for _ in range(n):
                nc.tensor.matmul(scratch, w1p, w1p, start=True, stop=True)

        tiles = {}
        mtiles = {}
        hs = {}  # pair index -> h SBUF tile
        E4s = {}  # block index -> E4 SBUF tile
        st = {"ph": None, "pg4": None, "po": None, "po_sb": None}

        def issue_group(g):
            xng = xnpool.tile([SUB, GROUP, KSUB, DW], F16, tag="xn")
            # 3:1 sync/scalar split keeps both HWDGE rings near-equal bytes
            eng = nc.scalar if g % 4 == 3 else nc.sync
            eng.dma_start(xng, xn_d[g])
            xtg = xtpool.tile([D, GROUP, SUPER], F8, tag="xt")
            nc.scalar.dma_start(xtg, xt_d[g])
            tiles[g] = (xng, xtg)
            if g % (MGROUP // GROUP) == 0:
                m_sb = mpool.tile([SUB, MGROUP, KSUB, NG], U8, tag="mask")
                nc.scalar.dma_start(m_sb, mask_d[g * GROUP // MGROUP])
                mtiles[g * GROUP // MGROUP] = m_sb

        def stage_a(b):
            """mm1 pairs + relu for the 4 supertiles of block b."""
            for j in range(EB):
                t = b * EB + j
                g, gi = divmod(t, GROUP)
                if gi == 1 and g + 4 < TG:
                    issue_group(g + 4)
                xtg = tiles[g][1]
                # packed mm1 batched over the RB=2 pair (N=512 per MM,
                # col-concurrent halves): ph[0:64,p,c] = h(node c of st p),
                # ph[64:128,p,c] = h(node 256+c of st p)
                if t % RB == 0:
                    st["ph"] = ps_h.tile([D, RB, 256], F32, name="ph")
                    ph = st["ph"]
                    nc.tensor.matmul(
                        ph[0:HID, :, :], w1p[:, 0:HID], xtg[:, gi : gi + RB, 0:256],
                        start=True, stop=True,
                    )
                    nc.tensor.matmul(
                        ph[HID:D, :, :], w1p[:, HID:D], xtg[:, gi : gi + RB, 256:512],
                        start=True, stop=True,
                    )
                    dummy(2)
                ph = st["ph"]
                if t % RB == RB - 1:
                    p = t // RB
                    h = hpool.tile([D, RB, 256], F16, name="h")
                    if p % 2 == 0:
                        nc.scalar.activation(
                            h, ph, mybir.ActivationFunctionType.Relu,
                            bias=b1d, scale=1.0,
                        )
                    else:
                        nc.vector.tensor_scalar(
                            h, ph, b1d, 0.0, mybir.AluOpType.add, mybir.AluOpType.max
                        )
                    hs[p] = h

        def stage_b(b):
            """mm2 quads + exp + E multiply for block b."""
            pg4 = ps_g.tile([SUB, EB, KSUB], F32, name="pg4")
            for j in range(EB):
                t = b * EB + j
                h = hs[t // RB]
                hj = t % RB
                for k in range(KSUB):
                    r0 = HID * (k // 2)
                    c0 = SUB * (k % 2)
                    nc.tensor.matmul(
                        pg4[:, j, k : k + 1],
                        h[r0 : r0 + HID, hj, c0 : c0 + SUB],
                        w2d[r0 : r0 + HID, :],
                        start=True,
                        stop=True,
                    )
                if hj == RB - 1:
                    del hs[t // RB]
            dummy(2)
            e4 = epool.tile([SUB, EB, KSUB], F32, tag="e")
            nc.scalar.activation(e4, pg4, mybir.ActivationFunctionType.Exp)
            t0 = b * EB
            m_sb = mtiles[t0 // MGROUP]
            E4 = epool.tile([SUB, EB, KSUB, NG], F16, tag="E")
            nc.gpsimd.tensor_mul(
                E4,
                m_sb[:, t0 % MGROUP : t0 % MGROUP + EB],
                e4.to_broadcast([SUB, EB, KSUB, NG]),
            )
            E4s[b] = E4

        pending_out = []

        def flush_out():
            for ob_idx, sb in pending_out:
                for s4 in range(KSUB):
                    nc.sync.dma_start(out_d[ob_idx, s4], sb[32 * s4 : 32 * s4 + NG])
            pending_out.clear()

        def stage_c(b):
            """po strip-matmuls + evacuation + (delayed) out DMA for block b."""
            flush_out()
            E4 = E4s.pop(b)
            for j in range(EB):
                t = b * EB + j
                g, gi = divmod(t, GROUP)
                xng = tiles[g][0]
                if t % PB == 0:
                    st["po"] = ps_o.tile([SUB, PB, DW], F32, name="po")
                po = st["po"]
                for k in range(KSUB):
                    nc.tensor.matmul(
                        po[32 * k : 32 * k + NG, t % PB, :],
                        E4[:, j, k, :],
                        xng[:, gi, k, :],
                        start=True,
                        stop=True,
                        tile_position=(0, 32 * k),
                    )
                if t % OB == 0:
                    st["po_sb"] = opool.tile([SUB, OB, DW], F16, tag="po", name="po_sb")
                po_sb = st["po_sb"]
                if t % PB == PB - 1:
                    s = (t % OB) - PB + 1
                    if (t // PB) % 2 == 0:
                        nc.vector.tensor_copy(po_sb[:, s : s + PB, :], po)
                    else:
                        nc.scalar.activation(
                            po_sb[:, s : s + PB, :],
                            po,
                            mybir.ActivationFunctionType.Copy,
                        )
                if t % PB == PB - 1:
                    dummy(2)
                if t % OB == OB - 1:
                    pending_out.append((t // OB, po_sb))
                if gi == GROUP - 1:
                    del tiles[g]

        for g0 in range(min(4, TG)):
            issue_group(g0)
        dummy(64)  # ~7us continuous PE activity: HAM warm before st 0

        blocks = T // EB
        for b in range(blocks + 2):
            if b < blocks:
                stage_a(b)
            if 1 <= b <= blocks:
                stage_b(b - 1)
            if b >= 2:
                stage_c(b - 2)
        flush_out()

    nc.compile()
    return nc


def preprocess(x: np.ndarray, batch: np.ndarray):
    """Shard + pad inputs, cast x to fp16 (natural) + fp8 (transposed)
    device layouts (grouped for batched DMA), build per-supertile masks
    and graph-id tables."""
    N = x.shape[0]
    n_core = -(-N // CORES)
    npc = -(-n_core // (SUPER * GROUP)) * (SUPER * GROUP)
    T = npc // SUPER
    TG = T // GROUP

    xs = np.zeros((CORES, npc, D), np.float32)
    b_pad = np.empty((CORES, npc), np.int64)
    valid = np.zeros((CORES, npc), bool)
    for c in range(CORES):
        s, e = c * n_core, min((c + 1) * n_core, N)
        n = e - s
        xs[c, :n] = x[s:e]
        b_pad[c, :n] = batch[s:e] if n > 0 else 0
        b_pad[c, n:] = batch[e - 1] if n > 0 else 0
        valid[c, :n] = True

    f16 = np.float16
    x16 = xs.astype(f16)  # [C, npc, D]
    # natural layout, grouped: [C, TG, SUB, GROUP, KSUB, DW]
    xn = np.zeros((CORES, TG, SUB, GROUP, KSUB, DW), f16)
    x6 = x16.reshape(CORES, TG, GROUP, KSUB, SUB, D).transpose(0, 1, 4, 2, 3, 5)
    xn[..., :D] = x6
    xn[..., D] = f16(1.0)
    # transposed gate layout in fp8, grouped: [C, TG, D, GROUP, SUPER]
    xt = np.ascontiguousarray(
        xs.astype(NP_F8).reshape(CORES, TG, GROUP, SUPER, D).transpose(0, 1, 4, 2, 3)
    )

    v = b_pad.reshape(CORES, T, SUPER)
    chg = np.zeros(v.shape, bool)
    chg[..., 1:] = v[..., 1:] != v[..., :-1]
    loc = np.cumsum(chg, axis=-1)  # [C,T,SUPER] local distinct index
    NG = int(loc.max()) + 1
    NG = max(4, -(-NG // 4) * 4)

    vmask = valid.reshape(CORES, T, SUPER)
    onehot = (loc[..., None] == np.arange(NG)) & vmask[..., None]
    # [C,T,SUPER,NG] -> [C, ceil(T/MGROUP), SUB, MGROUP, KSUB, NG]
    TM2 = -(-T // MGROUP)
    mask = np.zeros((CORES, TM2 * MGROUP, KSUB, SUB, NG), np.uint8)
    mask[:, :T] = onehot.reshape(CORES, T, KSUB, SUB, NG)
    mask = np.ascontiguousarray(
        mask.reshape(CORES, TM2, MGROUP, KSUB, SUB, NG).transpose(
            0, 1, 4, 2, 3, 5
        )
    )

    # pad nodes have all-zero mask rows (zero partials), so they may share
    # the last real graph's id slot without corrupting it
    gids = np.zeros((CORES, T, NG), np.int64)
    cc, tt = np.meshgrid(np.arange(CORES), np.arange(T), indexing="ij")
    cc = cc[..., None] * np.ones((1, 1, SUPER), int)
    tt = tt[..., None] * np.ones((1, 1, SUPER), int)
    gids[cc.ravel(), tt.ravel(), loc.ravel()] = v.ravel()

    return xn, xt, mask, gids, T, NG


def _kernel_impl(x, batch, W1, b1, W2, b2=None, **run_kwargs):
    f16 = np.float16
    x = np.ascontiguousarray(np.asarray(x, dtype=np.float32))
    batch = np.asarray(batch).astype(np.int64)
    W1 = np.asarray(W1, dtype=np.float32).astype(f16)  # [D, HID]
    b1 = np.asarray(b1, dtype=np.float32).reshape(HID, 1)
    W2 = np.asarray(W2, dtype=np.float32).astype(f16).reshape(HID, 1)
    w1p = np.concatenate([W1, W1], axis=1)  # [D, D]
    b1d = np.concatenate([b1, b1], axis=0)  # [D, 1]
    w2d = np.concatenate([W2, W2], axis=0)  # [D, 1]

    xn, xt, mask, gids, T, NG = preprocess(x, batch)

    nc = build_program(T, NG)
    in_maps = [
        {
            "xn": xn[c],
            "xt": xt[c],
            "mask": mask[c],
            "w1p": w1p,
            "b1d": b1d,
            "w2d": w2d,
        }
        for c in range(CORES)
    ]
    # The axon/TRN device occasionally comes up wedged from a prior run
    # (NRT exec errors that clear after a few attempts) -- retry rather
    # than fail the whole call.
    last_err = None
    for attempt in range(4):
        try:
            res = run_bass_kernel_spmd(
                nc, in_maps, core_ids=list(range(CORES)), **run_kwargs
            )
            break
        except Exception as err:  # noqa: BLE001 - device-side transients
            last_err = err
            if attempt == 3:
                raise
            if "nrt_profile" in str(err):
                run_kwargs = {**run_kwargs, "trace": False}
            else:
                time.sleep(20.0)
    # [C, T//OB, KSUB, NG, OB, DW] -> [C, T, KSUB, NG, DW]
    parts = np.stack([r["out_part"] for r in res.results]).astype(np.float32)
    C = parts.shape[0]
    parts = parts.transpose(0, 1, 4, 2, 3, 5).reshape(C, T, KSUB, NG, DW)

    G = G_SEGMENTS
    acc = np.zeros((G + 1, DW), np.float32)
    idx = np.where(gids >= 0, gids, G)  # [C, T, NG]
    idx = np.broadcast_to(idx[:, :, None, :], (C, T, KSUB, NG)).ravel()
    np.add.at(acc, idx, parts.reshape(-1, DW))
    den = acc[:G, D]
    S = acc[:G, :D]
    out = np.where(den[:, None] > 0, S / np.maximum(den, 1e-30)[:, None], 0.0)
    return out.astype(np.float32), res


def kernel(x, batch, W1, b1, W2, b2):
    out, _ = _kernel_impl(x, batch, W1, b1, W2, b2)
    return out
